# revision 2
# baseline (speedup 1.0000x reference)
"""Trainium2 Bass kernel for nn_ANN_Comp_29240137351521 (dense_cnn).

Reference computes, per batch row b of x [16384, 512] (complex, given as
real/imag f32 pairs):
    h = x @ w0                      # [B, 512] complex
    a = ifft(fft(h, n=1023)^2)      # full self-convolution, [B, 1023]
    out = |a @ wlast|               # [B, 10] f32

Algebraic collapse used here: the self-convolution + final contraction is a
polynomial-evaluation identity. With L = 1024 >= 2*512-1 evaluation points at
the L-th roots of unity:
    e   = x @ F        where F  = fft(w0, n=L, axis=1)        [512, L]
    z   = (e*e) @ Wt   where Wt = ifft(pad(wlast, L), axis=0) [L, 10]
    out = |z|
so the whole network is two dense matmuls + an elementwise complex square --
no FFT on device. F and Wt are tiny weight transforms folded on the host.

Real-expanded form computed on device (per core, data-parallel over batch):
    er = xr@Fr - xi@Fi ;  ei = xr@Fi + xi@Fr          (PSUM accumulation)
    sr = er^2 - ei^2   ;  w  = er*ei                  (ACT squares + DVE)
    [zr | zi] = sr @ [Wtr|Wti] + w @ [-2Wti|2Wtr]     (stacked second matmul)
    host: out = sqrt(zr^2 + zi^2)

Everything runs transposed (l on partitions, batch on the free axis) so the
second matmul needs no on-device transpose; x is fed pre-transposed from host.
Matmuls use float32r (full-rate fp32 on the PE at N>=256).

Sharding: pure data parallel -- batch split 8 ways, weights replicated.
"""

import numpy as np

import concourse.bass as bass
import concourse.mybir as mybir
from concourse import bacc, tile
from concourse.bass_utils import run_bass_kernel_spmd

NCORES = 8
B, D, L, C = 16384, 512, 1024, 10
BC = B // NCORES          # batch per core = 2048
P = 128                   # partitions
BN = 512                  # batch columns per PSUM tile (fp32 moving-max / bank)
ND = D // P               # 4 contraction chunks
NL = L // P               # 8 output-l chunks
NB = BC // BN             # 4 batch positions per core

F32 = mybir.dt.float32
F32R = mybir.dt.float32r

_NC_CACHE = None


def build_nc():
    """Build (once) the single-core Bass graph; SPMD-replicated to 8 cores."""
    global _NC_CACHE
    if _NC_CACHE is not None:
        return _NC_CACHE

    nc = bacc.Bacc(None, target_bir_lowering=False)

    xtr_d = nc.declare_dram_parameter("xT_r", [D, BC], F32R, isOutput=False)
    xti_d = nc.declare_dram_parameter("xT_i", [D, BC], F32R, isOutput=False)
    fr_d = nc.declare_dram_parameter("F_r", [D, L], F32R, isOutput=False)
    fi_d = nc.declare_dram_parameter("F_i", [D, L], F32R, isOutput=False)
    wa_d = nc.declare_dram_parameter("WtA", [L, 2 * C], F32R, isOutput=False)
    wb_d = nc.declare_dram_parameter("WtB", [L, 2 * C], F32R, isOutput=False)
    out_d = nc.declare_dram_parameter("out", [2 * C, BC], F32, isOutput=True)

    with tile.TileContext(nc) as tc:
        with (
            tc.tile_pool(name="wts", bufs=1) as wts,
            tc.tile_pool(name="xs", bufs=1) as xs,
            tc.tile_pool(name="tmp", bufs=3) as tmp,
            tc.tile_pool(name="sq", bufs=3) as sq,
            tc.tile_pool(name="zo", bufs=2) as zo,
            tc.tile_pool(name="pse", bufs=2, space="PSUM") as pse,
            tc.tile_pool(name="psz", bufs=2, space="PSUM") as psz,
        ):
            # --- resident weights -------------------------------------------
            fr, fi, fin = [], [], []
            for d in range(ND):
                t = wts.tile([P, L], F32R, tag=f"fr{d}")
                nc.sync.dma_start(t[:], fr_d[d * P:(d + 1) * P, :])
                fr.append(t)
                t = wts.tile([P, L], F32R, tag=f"fi{d}")
                nc.sync.dma_start(t[:], fi_d[d * P:(d + 1) * P, :])
                fi.append(t)
            for d in range(ND):
                t = wts.tile([P, L], F32R, tag=f"fin{d}")
                nc.scalar.mul(t[:], fi[d][:], -1.0)   # -F_i for er accumulation
                fin.append(t)
            wa, wb = [], []
            for l in range(NL):
                t = wts.tile([P, 2 * C], F32R, tag=f"wa{l}")
                nc.sync.dma_start(t[:], wa_d[l * P:(l + 1) * P, :])
                wa.append(t)
                t = wts.tile([P, 2 * C], F32R, tag=f"wb{l}")
                nc.sync.dma_start(t[:], wb_d[l * P:(l + 1) * P, :])
                wb.append(t)

            # --- resident transposed activations ----------------------------
            xtr, xti = [], []
            for d in range(ND):
                t = xs.tile([P, BC], F32R, tag=f"xtr{d}")
                nc.sync.dma_start(t[:], xtr_d[d * P:(d + 1) * P, :])
                xtr.append(t)
                t = xs.tile([P, BC], F32R, tag=f"xti{d}")
                nc.sync.dma_start(t[:], xti_d[d * P:(d + 1) * P, :])
                xti.append(t)

            # --- main pipeline ----------------------------------------------
            for b in range(NB):
                bs = slice(b * BN, (b + 1) * BN)
                zz = psz.tile([2 * C, BN], F32, tag="zz")
                for l in range(NL):
                    ls = slice(l * P, (l + 1) * P)
                    er = pse.tile([P, BN], F32, tag="er")
                    ei = pse.tile([P, BN], F32, tag="ei")
                    # er = xr@Fr - xi@Fi   (transposed: F chunk is lhsT)
                    for d in range(ND):
                        nc.tensor.matmul(
                            er[:], fr[d][:, ls],
                            xtr[d][:, bs],
                            start=(d == 0), stop=False)
                    for d in range(ND):
                        nc.tensor.matmul(
                            er[:], fin[d][:, ls],
                            xti[d][:, bs],
                            start=False, stop=(d == ND - 1))
                    # ei = xr@Fi + xi@Fr
                    for d in range(ND):
                        nc.tensor.matmul(
                            ei[:], fi[d][:, ls],
                            xtr[d][:, bs],
                            start=(d == 0), stop=False)
                    for d in range(ND):
                        nc.tensor.matmul(
                            ei[:], fr[d][:, ls],
                            xti[d][:, bs],
                            start=False, stop=(d == ND - 1))

                    # squares: u = er^2, v = ei^2 on ACT; sr = u-v, w = er*ci
                    u = tmp.tile([P, BN], F32, tag="u")
                    nc.scalar.square(u[:], er[:])
                    v = tmp.tile([P, BN], F32, tag="v")
                    nc.scalar.square(v[:], ei[:])
                    ci = tmp.tile([P, BN], F32, tag="ci")
                    nc.scalar.copy(ci[:], ei[:])
                    sr = sq.tile([P, BN], F32R, tag="sr")
                    nc.vector.tensor_sub(sr[:], u[:], v[:])
                    w = sq.tile([P, BN], F32R, tag="w")
                    nc.vector.tensor_mul(w[:], er[:], ci[:])

                    # z accumulation: zz += WtA[l].T@sr + WtB[l].T@w
                    nc.tensor.matmul(
                        zz[:], wa[l][:], sr[:],
                        start=(l == 0), stop=False, skip_group_check=True)
                    nc.tensor.matmul(
                        zz[:], wb[l][:], w[:],
                        start=False, stop=(l == NL - 1), skip_group_check=True)

                zt = zo.tile([2 * C, BN], F32, tag="zt")
                nc.scalar.copy(zt[:], zz[:])
                nc.sync.dma_start(out_d[:, bs], zt[:])

    nc.compile()
    _NC_CACHE = nc
    return nc


def _host_weights(w0_real, w0_imag, wlast_real, wlast_imag):
    w0 = w0_real.astype(np.float64) + 1j * w0_imag.astype(np.float64)
    wl = wlast_real.astype(np.float64) + 1j * wlast_imag.astype(np.float64)
    F = np.fft.fft(w0, n=L, axis=1)                       # [512, 1024]
    Wt = np.fft.ifft(
        np.concatenate([wl, np.zeros((1, C))], axis=0), axis=0)  # [1024, 10]
    Fr = np.ascontiguousarray(F.real, dtype=np.float32)
    Fi = np.ascontiguousarray(F.imag, dtype=np.float32)
    Wtr, Wti = Wt.real, Wt.imag
    WtA = np.ascontiguousarray(np.hstack([Wtr, Wti]), dtype=np.float32)
    WtB = np.ascontiguousarray(np.hstack([-2 * Wti, 2 * Wtr]), dtype=np.float32)
    return Fr, Fi, WtA, WtB


def make_in_maps(x_real, x_imag, w0_real, w0_imag, wlast_real, wlast_imag):
    Fr, Fi, WtA, WtB = _host_weights(w0_real, w0_imag, wlast_real, wlast_imag)
    in_maps = []
    for c in range(NCORES):
        sl = slice(c * BC, (c + 1) * BC)
        in_maps.append({
            "xT_r": np.ascontiguousarray(x_real[sl].T),
            "xT_i": np.ascontiguousarray(x_imag[sl].T),
            "F_r": Fr, "F_i": Fi, "WtA": WtA, "WtB": WtB,
        })
    return in_maps


def postprocess(results):
    """results: list of per-core dicts with 'out' [20, BC] -> [B, C] f32."""
    outs = []
    for c in range(NCORES):
        o = results[c]["out"]
        mag = np.sqrt(o[:C] ** 2 + o[C:2 * C] ** 2).T     # [BC, 10]
        outs.append(mag)
    return np.ascontiguousarray(np.concatenate(outs, axis=0), dtype=np.float32)


def kernel(x_real, x_imag, w0_real, w0_imag, wlast_real, wlast_imag):
    nc = build_nc()
    in_maps = make_in_maps(
        x_real, x_imag, w0_real, w0_imag, wlast_real, wlast_imag)
    res = run_bass_kernel_spmd(nc, in_maps, core_ids=list(range(NCORES)))
    return postprocess(res.results)


# revision 4
# speedup vs baseline: 1.0597x; 1.0597x over previous
"""Trainium2 Bass kernel for nn_ANN_Comp_29240137351521 (dense_cnn).

Reference computes, per batch row b of x [16384, 512] (complex, given as
real/imag f32 pairs):
    h = x @ w0                      # [B, 512] complex
    a = ifft(fft(h, n=1023)^2)      # full self-convolution, [B, 1023]
    out = |a @ wlast|               # [B, 10] f32

Algebraic collapse used here: the self-convolution + final contraction is a
polynomial-evaluation identity. With L = 1024 >= 2*512-1 evaluation points at
the L-th roots of unity:
    e   = x @ F        where F  = fft(w0, n=L, axis=1)        [512, L]
    z   = (e*e) @ Wt   where Wt = ifft(pad(wlast, L), axis=0) [L, 10]
    out = |z|
so the whole network is two dense matmuls + an elementwise complex square --
no FFT on device. F and Wt are tiny weight transforms folded on the host.

Real-expanded form computed on device (per core, data-parallel over batch):
    er = xr@Fr - xi@Fi ;  ei = xr@Fi + xi@Fr          (PSUM accumulation)
    sr = er^2 - ei^2   ;  w  = er*ei                  (ACT squares + DVE)
    [zr | zi] = sr @ [Wtr|Wti] + w @ [-2Wti|2Wtr]     (stacked second matmul)
    host: out = sqrt(zr^2 + zi^2)

Everything runs transposed (l on partitions, batch on the free axis) so the
second matmul needs no on-device transpose; x is fed pre-transposed from the
host in bf16 (measured end-to-end error 4e-3 of output scale, ~5x under the
2e-2 gate; squares/accumulations stay fp32).

Sharding: pure data parallel -- batch split 8 ways, weights replicated.
"""

import numpy as np
import ml_dtypes

import concourse.bass as bass
import concourse.mybir as mybir
from concourse import bacc, tile
from concourse.bass_utils import run_bass_kernel_spmd

NCORES = 8
B, D, L, C = 16384, 512, 1024, 10
BC = B // NCORES          # batch per core = 2048
P = 128                   # partitions
BN = 512                  # batch columns per PSUM tile
ND = D // P               # 4 contraction chunks
NL = L // P               # 8 output-l chunks
NB = BC // BN             # 4 batch positions per core

F32 = mybir.dt.float32
BF16 = mybir.dt.bfloat16

_NC_CACHE = None


def build_nc():
    """Build (once) the single-core Bass graph; SPMD-replicated to 8 cores."""
    global _NC_CACHE
    if _NC_CACHE is not None:
        return _NC_CACHE

    nc = bacc.Bacc(None, target_bir_lowering=False)

    xtr_d = nc.declare_dram_parameter("xT_r", [D, BC], BF16, isOutput=False)
    xti_d = nc.declare_dram_parameter("xT_i", [D, BC], BF16, isOutput=False)
    fr_d = nc.declare_dram_parameter("F_r", [D, L], BF16, isOutput=False)
    fi_d = nc.declare_dram_parameter("F_i", [D, L], BF16, isOutput=False)
    fin_d = nc.declare_dram_parameter("F_in", [D, L], BF16, isOutput=False)
    wa_d = nc.declare_dram_parameter("WtA", [L, 2 * C], BF16, isOutput=False)
    wb_d = nc.declare_dram_parameter("WtB", [L, 2 * C], BF16, isOutput=False)
    out_d = nc.declare_dram_parameter("out", [2 * C, BC], F32, isOutput=True)

    with tile.TileContext(nc) as tc:
        with (
            tc.tile_pool(name="wts", bufs=1) as wts,
            tc.tile_pool(name="xs", bufs=2) as xs,
            tc.tile_pool(name="tmp", bufs=3) as tmp,
            tc.tile_pool(name="sq", bufs=3) as sq,
            tc.tile_pool(name="zo", bufs=2) as zo,
            tc.tile_pool(name="pse", bufs=2, space="PSUM") as pse,
            tc.tile_pool(name="psz", bufs=2, space="PSUM") as psz,
        ):
            # --- resident weights (DMAs spread over two queues) -------------
            wa, wb = [], []
            for l in range(NL):
                t = wts.tile([P, 2 * C], BF16, tag=f"wa{l}")
                nc.sync.dma_start(t[:], wa_d[l * P:(l + 1) * P, :])
                wa.append(t)
                t = wts.tile([P, 2 * C], BF16, tag=f"wb{l}")
                nc.sync.dma_start(t[:], wb_d[l * P:(l + 1) * P, :])
                wb.append(t)
            fr, fi, fin = [], [], []
            for d in range(ND):
                t = wts.tile([P, L], BF16, tag=f"fr{d}")
                nc.sync.dma_start(t[:], fr_d[d * P:(d + 1) * P, :])
                fr.append(t)
                t = wts.tile([P, L], BF16, tag=f"fi{d}")
                nc.gpsimd.dma_start(t[:], fi_d[d * P:(d + 1) * P, :])
                fi.append(t)
                t = wts.tile([P, L], BF16, tag=f"fin{d}")
                nc.scalar.dma_start(t[:], fin_d[d * P:(d + 1) * P, :])
                fin.append(t)

            # --- main pipeline: stream x per batch position -----------------
            for b in range(NB):
                bs = slice(b * BN, (b + 1) * BN)
                xtr, xti = [], []
                for d in range(ND):
                    t = xs.tile([P, BN], BF16, tag=f"xtr{d}")
                    eng = nc.sync if d % 2 == 0 else nc.gpsimd
                    eng.dma_start(t[:], xtr_d[d * P:(d + 1) * P, bs])
                    xtr.append(t)
                    t = xs.tile([P, BN], BF16, tag=f"xti{d}")
                    eng = nc.gpsimd if d % 2 == 0 else nc.sync
                    eng.dma_start(t[:], xti_d[d * P:(d + 1) * P, bs])
                    xti.append(t)

                zz = psz.tile([2 * C, BN], F32, tag="zz")
                for l in range(NL):
                    ls = slice(l * P, (l + 1) * P)
                    er = pse.tile([P, BN], F32, tag="er")
                    ei = pse.tile([P, BN], F32, tag="ei")
                    # fr[d] serves two matmuls back-to-back (weight reuse):
                    #   er += fr.T @ xtr ;  ei += fr.T @ xti
                    for d in range(ND):
                        nc.tensor.matmul(
                            er[:], fr[d][:, ls], xtr[d][:],
                            start=(d == 0), stop=False,
                            skip_group_check=True)
                        nc.tensor.matmul(
                            ei[:], fr[d][:, ls], xti[d][:],
                            start=(d == 0), stop=False,
                            skip_group_check=True)
                    # ei += fi.T @ xtr
                    for d in range(ND):
                        nc.tensor.matmul(
                            ei[:], fi[d][:, ls], xtr[d][:],
                            start=False, stop=(d == ND - 1),
                            skip_group_check=True)
                    # er += (-fi).T @ xti
                    for d in range(ND):
                        nc.tensor.matmul(
                            er[:], fin[d][:, ls], xti[d][:],
                            start=False, stop=(d == ND - 1),
                            skip_group_check=True)

                    # squares: u = er^2, v = ei^2, ci = ei on ACT;
                    # sr = u - v, w = er * ci on DVE (bf16 outputs)
                    u = tmp.tile([P, BN], F32, tag="u")
                    nc.scalar.square(u[:], er[:])
                    v = tmp.tile([P, BN], F32, tag="v")
                    nc.scalar.square(v[:], ei[:])
                    ci = tmp.tile([P, BN], F32, tag="ci")
                    nc.scalar.copy(ci[:], ei[:])
                    sr = sq.tile([P, BN], BF16, tag="sr")
                    nc.vector.tensor_sub(sr[:], u[:], v[:])
                    w = sq.tile([P, BN], BF16, tag="w")
                    nc.vector.tensor_mul(w[:], er[:], ci[:])

                    # z accumulation: zz += WtA[l].T@sr + WtB[l].T@w
                    nc.tensor.matmul(
                        zz[:], wa[l][:], sr[:],
                        start=(l == 0), stop=False, skip_group_check=True)
                    nc.tensor.matmul(
                        zz[:], wb[l][:], w[:],
                        start=False, stop=(l == NL - 1), skip_group_check=True)

                zt = zo.tile([2 * C, BN], F32, tag="zt")
                nc.scalar.copy(zt[:], zz[:])
                nc.sync.dma_start(out_d[:, bs], zt[:])

    nc.compile()
    _NC_CACHE = nc
    return nc


def _host_weights(w0_real, w0_imag, wlast_real, wlast_imag):
    w0 = w0_real.astype(np.float64) + 1j * w0_imag.astype(np.float64)
    wl = wlast_real.astype(np.float64) + 1j * wlast_imag.astype(np.float64)
    F = np.fft.fft(w0, n=L, axis=1)                       # [512, 1024]
    Wt = np.fft.ifft(
        np.concatenate([wl, np.zeros((1, C))], axis=0), axis=0)  # [1024, 10]
    bf = ml_dtypes.bfloat16
    Fr = np.ascontiguousarray(F.real, dtype=bf)
    Fi = np.ascontiguousarray(F.imag, dtype=bf)
    Fin = np.ascontiguousarray(-F.imag, dtype=bf)
    Wtr, Wti = Wt.real, Wt.imag
    WtA = np.ascontiguousarray(np.hstack([Wtr, Wti]), dtype=bf)
    WtB = np.ascontiguousarray(np.hstack([-2 * Wti, 2 * Wtr]), dtype=bf)
    return Fr, Fi, Fin, WtA, WtB


def make_in_maps(x_real, x_imag, w0_real, w0_imag, wlast_real, wlast_imag):
    Fr, Fi, Fin, WtA, WtB = _host_weights(
        w0_real, w0_imag, wlast_real, wlast_imag)
    bf = ml_dtypes.bfloat16
    xr = np.ascontiguousarray(x_real.T, dtype=bf)   # [512, 16384]
    xi = np.ascontiguousarray(x_imag.T, dtype=bf)
    in_maps = []
    for c in range(NCORES):
        sl = slice(c * BC, (c + 1) * BC)
        in_maps.append({
            "xT_r": np.ascontiguousarray(xr[:, sl]),
            "xT_i": np.ascontiguousarray(xi[:, sl]),
            "F_r": Fr, "F_i": Fi, "F_in": Fin, "WtA": WtA, "WtB": WtB,
        })
    return in_maps


def postprocess(results):
    """results: list of per-core dicts with 'out' [20, BC] -> [B, C] f32."""
    outs = []
    for c in range(NCORES):
        o = results[c]["out"]
        mag = np.sqrt(o[:C] ** 2 + o[C:2 * C] ** 2).T     # [BC, 10]
        outs.append(mag)
    return np.ascontiguousarray(np.concatenate(outs, axis=0), dtype=np.float32)


def kernel(x_real, x_imag, w0_real, w0_imag, wlast_real, wlast_imag):
    nc = build_nc()
    in_maps = make_in_maps(
        x_real, x_imag, w0_real, w0_imag, wlast_real, wlast_imag)
    res = run_bass_kernel_spmd(nc, in_maps, core_ids=list(range(NCORES)))
    return postprocess(res.results)
